# revision 2
# baseline (speedup 1.0000x reference)
"""PointConvolution (8-neighbor shifted diffs + 1x1 conv) as a single 3x3 conv,
run data-parallel across 8 TRN2 NeuronCores via Bass/Tile.

Math: out[o,h,w] = sum_k sum_c W[o,3k+c] * (xpad[c,h+ik,w+jk] - x[c,h,w]) + b[o]
    = sum_{c,i,j} K3[o,c,i,j] * xpad[c,h+i,w+j] + b[o]
  where K3 gets W at the 8 non-center taps and -sum(W over taps) at center.

Device scheme per core (2 images), v7 (fp16 I/O, host-built j-replicas):
  - Output rows in chunks of 32 (TB=8 groups of G=4 rows).
  - M=128 PSUM partitions = (g in 0..3, o in 0..31); contraction partitions
    18j + 6c + s for kernel column j, channel c, window row s in 0..5.
  - Host pre-gathers each chunk's input window into [54, TB*Wp] fp16 im2row
    (rows AND the three j column-shifts are materialized host-side, so the
    device does no replica copies and needs no 32-aligned gap partitions).
  - Per group t: ONE self-loading fp16 matmul [54x128]x[54x512] (start+stop),
    accumulating in fp32 PSUM.
  - PSUM -> SBUF drain: one DVE tensor_scalar_add per 4-bank half (adds fp32
    bias, converts to fp16), then one 128x8KB-descriptor DMA per chunk writes
    a permuted fp16 DRAM layout; host transposes + upcasts during unshard.
  - HBM traffic/core: 14.2MB in + 33.5MB out = 47.7MB -> ~133us roofline at
    358 GB/s (vs 76.6MB/~214us for the fp32 v6). fp16 rounding keeps rel err
    ~1e-3, far under the 2e-2 gate.
"""

import numpy as np

import concourse.bacc as bacc
import concourse.bass as bass
import concourse.tile as tile
from concourse import mybir
from concourse.bass_utils import run_bass_kernel_spmd

# Problem constants (hardcoded per harness contract)
B, C, H, W_DIM, OUT = 16, 3, 512, 512, 32
KS, P = 3, 1
NCORES = 8
NB = B // NCORES          # images per core = 2
Hp, Wp = H + 2 * P, W_DIM + 2 * P   # 514, 514

G = 4                     # output rows per matmul group
S = G + KS - 1            # input rows per group window = 6
T = 4                     # groups per PSUM half
TB = 8                    # groups per chunk (32 output rows)
CH = G * TB               # 32 output rows per chunk
NCHUNK = H // CH          # 16 chunks per image
K = KS * C * S            # 54 contraction partitions (j-replicas built on host)
M = G * OUT               # 128 output partitions
FW = TB * Wp              # 4112 free cols per contraction row
OBF = 2 * T * W_DIM       # 4096 free cols in the output tile

F32 = mybir.dt.float32
F16 = mybir.dt.float16


def _coords():
    i, j = np.meshgrid(np.arange(KS), np.arange(KS))
    coords = np.dstack((i.reshape(-1), j.reshape(-1)))[0]
    return coords[np.any(coords != P, axis=1)]


def _build_weights(W, b):
    K3 = np.zeros((OUT, C, KS, KS), np.float32)
    Wr = W.reshape(OUT, 8, C)
    for k, (i, j) in enumerate(_coords()):
        K3[:, :, i, j] += Wr[:, k, :]
    K3[:, :, P, P] = -Wr.sum(axis=1)

    # wt[18j + 6c + s, 32g + o] = K3[o, c, s-g, j] when 0 <= s-g < KS
    wt = np.zeros((K, M), np.float32)
    for j in range(KS):
        for c in range(C):
            for s in range(S):
                for g in range(G):
                    i = s - g
                    if 0 <= i < KS:
                        wt[S * C * j + S * c + s, OUT * g: OUT * (g + 1)] = K3[:, c, i, j]
    bias = np.tile(b.astype(np.float32), G).reshape(M, 1)
    return wt.astype(np.float16), bias


def _build_xin(x):
    """[B,C,H,W] fp32 -> [B, NCHUNK, K, TB*Wp] fp16 im2row over rows, with the
    three j column-shift replicas stacked on the partition axis (padding
    embedded; 2 extra zero cols on the right so j-shifts never run off)."""
    x16 = np.ascontiguousarray(x, np.float32).astype(np.float16)
    xpad = np.pad(x16, ((0, 0), (0, 0), (P, P), (P, P + 2)))  # [B,C,514,516]
    ch = np.arange(NCHUNK)[:, None, None]
    s = np.arange(S)[None, :, None]
    t = np.arange(TB)[None, None, :]
    rows = CH * ch + G * t + s                      # [NCHUNK, S, TB]
    out = np.empty((B, NCHUNK, KS, C, S, TB, Wp), np.float16)
    for j in range(KS):
        rep = xpad[:, :, :, j:j + Wp]               # [B,C,514,514]
        big = rep[:, :, rows, :]                    # [B, C, NCHUNK, S, TB, Wp]
        out[:, :, j] = big.transpose(0, 2, 1, 3, 4, 5)
    return out.reshape(B, NCHUNK, K, FW)


def _build_bass():
    # Bacc (not plain Bass): its compile() runs move_matmul_waits_to_ldweights
    # and generate_event_semaphores, required because TRN2 instructions take
    # at most one semaphore wait.
    nc = bacc.Bacc("TRN2")
    x_d = nc.declare_dram_parameter("xin", [NB, NCHUNK, K, FW], F16, isOutput=False)
    wt_d = nc.declare_dram_parameter("wt", [K, M], F16, isOutput=False)
    b_d = nc.declare_dram_parameter("bias", [M, 1], F32, isOutput=False)
    out_d = nc.declare_dram_parameter("out", [NB, NCHUNK, M, OBF], F16, isOutput=True)

    with tile.TileContext(nc) as tc:
        with (
            tc.tile_pool(name="wpool", bufs=1) as wpool,
            tc.tile_pool(name="xpool", bufs=3) as xpool,
            tc.tile_pool(name="opool", bufs=3) as opool,
            tc.tile_pool(name="psum", bufs=2, space=bass.MemorySpace.PSUM) as ppool,
        ):
            wsb = wpool.tile([K, M], F16)
            nc.scalar.dma_start(wsb[:], wt_d[:])
            bsb = wpool.tile([M, 1], F32)
            nc.scalar.dma_start(bsb[:], b_d[:])

            for n in range(NB):
                for chunk in range(NCHUNK):
                    xin = xpool.tile([K, FW], F16)
                    src = bass.AP(
                        x_d,
                        (n * NCHUNK + chunk) * K * FW,
                        [[FW, K], [1, FW]],
                    )
                    nc.gpsimd.dma_start(xin[:], src)

                    ob = opool.tile([M, OBF], F16)
                    for half in range(2):
                        ps = ppool.tile([M, T, W_DIM], F32)
                        for t4 in range(T):
                            t = half * T + t4
                            nc.tensor.matmul(
                                ps[:, t4, :],
                                wsb[:],
                                xin[:, Wp * t: Wp * t + W_DIM],
                                start=True,
                                stop=True,
                            )
                        nc.vector.tensor_scalar_add(
                            ob[:, half * T * W_DIM: (half + 1) * T * W_DIM],
                            ps[:, :, :],
                            bsb[:],
                        )

                    dst = bass.AP(
                        out_d,
                        (n * NCHUNK + chunk) * M * OBF,
                        [[OBF, M], [1, OBF]],
                    )
                    nc.sync.dma_start(dst, ob[:])
    nc.finalize()
    return nc


_NC_CACHE = None


def _get_nc():
    global _NC_CACHE
    if _NC_CACHE is None:
        _NC_CACHE = _build_bass()
    return _NC_CACHE


def kernel(x, W, b, trace=False, **trace_kw):
    xin = _build_xin(np.asarray(x, np.float32))
    wt, bias = _build_weights(np.asarray(W, np.float32), np.asarray(b, np.float32))
    in_maps = [
        {"xin": xin[NB * m: NB * (m + 1)], "wt": wt, "bias": bias}
        for m in range(NCORES)
    ]
    res = run_bass_kernel_spmd(
        _get_nc(), in_maps, list(range(NCORES)), trace=trace, **trace_kw
    )
    # Device layout [NB, NCHUNK, 32g+o, (half,t4,w)] -> [B, OUT, H, W]:
    # row = CH*chunk + 4*(4*half + t4) + g
    parts = []
    for m in range(NCORES):
        o = res.results[m]["out"].reshape(NB, NCHUNK, G, OUT, 2, T, W_DIM)
        parts.append(o.transpose(0, 3, 1, 4, 5, 2, 6).reshape(NB, OUT, H, W_DIM))
    out = np.ascontiguousarray(np.concatenate(parts, axis=0)).astype(np.float32)
    if trace:
        kernel.last_results = res
    return out
